# revision 18
# baseline (speedup 1.0000x reference)
"""Bass/Trainium2 kernel for nn_CGRE_68719477510 (ragged_sequence).

Restructure: scores[i] = X[i] . Constraints[rel(bag(i))] and the classifier
out = bag @ W.T are both projections of X onto small [53, 2070] matrices.
So one device pass computes Y = [Constraints; W] @ X.T  ([106, N]) — the only
traffic proportional to X. The segment softmax + weighted sum then operate on
the projected [N, 53] rows (P = X @ W.T), never touching X again:
    out[bag] = sum_i softmax_i(S) * P[i]  ==  (sum_i w_i X_i) @ W.T
Sharding: split sentences N=65536 into 8 contiguous chunks of 8192 (one per
core); replicate the small combined weight. The ragged segment ops run on
host over the tiny [N, 53] projection.

Precision: X and [C; W] are shipped in fp16 (e5m10). fp16xfp16 products are
exact in the f32 PSUM accumulator, so the only noise is the input rounding
(~2^-11 relative), giving ~1.7e-3 final Frobenius error — well under the
2e-2 gate — at half the DMA traffic of an f32/bf16-pair encoding.

DMA shape: the HWDGE rings release roughly one trigger per completed
transfer (~1-2 us apart), so sustained bandwidth ~ bytes-per-trigger /
pacing — triggers must stay fat. The host packs the first 16 k-chunks of
each column block k-interleaved so ONE dma_start fetches a whole block
across all k as a contiguous [128, 16*w] slice (16 KB descriptors, ~2 MB
per trigger). Column blocks shrink toward the end (512...128) and the
final block is k-split so only ~128 columns of matmul+copy+writeback trail
the last X byte. The ragged k=16 chunk (22 rows) rides in one early tile.
"""

import sys

sys.path.insert(0, "/opt/trn_rl_repo")

import numpy as np

N_SENT = 65536
D_FEAT = 2070
N_REL = 53
N_CORES = 8
N_PER_CORE = N_SENT // N_CORES  # 8192
M_OUT = 2 * N_REL  # 106 rows: [Constraints; W]

KC = 128                      # contraction chunk (partition dim)
NKF = 16                      # full 128-row k chunks (plus one 22-row tail)
N_KCHUNKS = NKF + 1           # 17
K_TAIL = D_FEAT - NKF * KC    # 22

# column blocks per core; fat 1024-col blocks carry the bulk (16 KB dma
# descriptors per half, fewer per-block PE transition penalties), the tail
# shrinks so the post-stream chain covers only 128 columns. b2 is 512 so
# the k16 tile slots into the sync ring at the PE-start sweet spot.
BLOCKS = [1024, 1024, 1024, 1024, 1024, 1024, 1024, 512, 256, 128, 128]
assert sum(BLOCKS) == N_PER_CORE
NW1 = 8  # blocks 0..7 merge into one fat writeback, the rest into a second
MM_G = 512  # psum accumulation group width (one PSUM bank)

_CACHE = {}


def _build_fp16():
    import concourse.mybir as mybir
    from concourse import bacc
    from concourse.tile import TileContext

    F16 = mybir.dt.float16
    F32 = mybir.dt.float32

    nc = bacc.Bacc("TRN2", target_bir_lowering=False, debug=True)
    # k-interleaved bulk: xfat[p, 16*cum(b) + k*w + c] = X[128k+p, c0+c]
    xfat = nc.dram_tensor("xfat", [KC, NKF * N_PER_CORE], F16, kind="ExternalInput")
    # ragged k=16 chunk, plain layout
    xk16 = nc.dram_tensor("xk16", [K_TAIL, N_PER_CORE], F16, kind="ExternalInput")
    # weights packed on host: cwf[p, k*106+m] = CW[m, 128k+p] (zero-padded)
    cwf = nc.dram_tensor("cwf", [KC, N_KCHUNKS * M_OUT], F16, kind="ExternalInput")
    yt = nc.dram_tensor("yt", [M_OUT, N_PER_CORE], F16, kind="ExternalOutput")

    X_BUFS = 5

    starts = [sum(BLOCKS[:i]) for i in range(len(BLOCKS))]
    fstarts = [NKF * s for s in starts]

    with TileContext(nc) as tc:
        with (
            tc.tile_pool(name="w", bufs=1) as wpool,
            tc.tile_pool(name="k16", bufs=1) as kpool,
            tc.tile_pool(name="x", bufs=X_BUFS) as xpool,
            tc.tile_pool(name="out", bufs=1) as opool,
            tc.tile_pool(name="psum", bufs=4, space="PSUM") as ppool,
        ):
            # weights ride gpsimd SWDGE during the ramp so the fat HWDGE
            # stream carries only bulk X
            wt = wpool.tile([KC, N_KCHUNKS * M_OUT], F16, tag="w")
            nc.gpsimd.dma_start(out=wt[:, :], in_=cwf[:, :])
            k16t = kpool.tile([K_TAIL, N_PER_CORE], F16, tag="k16")

            # Each block is split in half across BOTH rings (k0-7 on sync,
            # k8-15 on scalar) so blocks arrive in uniform lockstep at one
            # block per ~5.2us instead of pairwise bursts — the PE then never
            # stalls mid-stream (each stall resets the clock ramp to 1.2GHz
            # for ~3us). The shared k16 tile is slotted mid-ring so it lands
            # ~36us in; every block's accumulation does its k16 matmul FIRST,
            # delaying the PE start until then: the PE runs gapless at full
            # p-state from there and finishes right behind the last block.
            xts = {}
            for bi, (f0, w) in enumerate(zip(fstarts, BLOCKS)):
                xt = xpool.tile([KC, NKF * 1024], F16, tag="x")
                half = NKF * w // 2
                nc.sync.dma_start(out=xt[:, :half], in_=xfat[:, f0 : f0 + half])
                nc.scalar.dma_start(
                    out=xt[:, half : NKF * w], in_=xfat[:, f0 + half : f0 + NKF * w]
                )
                xts[bi] = xt
                if bi == 1:
                    nc.sync.dma_start(out=k16t[:, :], in_=xk16[:, :])

            out_t = opool.tile([M_OUT, N_PER_CORE], F16, tag="out")

            for bi, (c0, w) in enumerate(zip(starts, BLOCKS)):
                xt = xts[bi]
                # each 512-col psum group accumulates in one PSUM bank; a
                # 1024-col dma block holds two groups
                for g0 in range(0, w, MM_G):
                    gw = min(MM_G, w - g0)
                    psum = ppool.tile([M_OUT, MM_G], F32, tag="ps")
                    ws = slice(NKF * M_OUT, N_KCHUNKS * M_OUT)
                    nc.tensor.matmul(
                        psum[:, :gw],
                        wt[:K_TAIL, ws],
                        k16t[:, c0 + g0 : c0 + g0 + gw],
                        start=True,
                        stop=False,
                    )
                    for k in range(NKF):
                        ws = slice(k * M_OUT, (k + 1) * M_OUT)
                        nc.tensor.matmul(
                            psum[:, :gw],
                            wt[:, ws],
                            xt[:, k * w + g0 : k * w + g0 + gw],
                            start=False,
                            stop=(k == NKF - 1),
                        )
                    nc.vector.tensor_copy(
                        out=out_t[:, c0 + g0 : c0 + g0 + gw], in_=psum[:, :gw]
                    )
                # SWDGE write descriptors only drain once the read stream
                # ends, and the SWDGE queue is in-order — so merge the
                # writeback into two FAT writes (few 15KB descriptors beat
                # many thin ones): the bulk once blocks 0..NW1-1 are cast,
                # and the small tail right after the final cast
                if bi == NW1 - 1:
                    wb = starts[NW1 - 1] + BLOCKS[NW1 - 1]
                    nc.gpsimd.dma_start(out=yt[:, :wb], in_=out_t[:, :wb])
                elif bi == len(BLOCKS) - 1:
                    wb = starts[NW1 - 1] + BLOCKS[NW1 - 1]
                    nc.gpsimd.dma_start(out=yt[:, wb:], in_=out_t[:, wb:])

    nc.compile()
    return nc


def _build():
    if "fp16" not in _CACHE:
        _CACHE["fp16"] = _build_fp16()
    return _CACHE["fp16"]


def _pack_weights(CWT, dtype):
    """CWT [D_FEAT, 106] -> [128, 17*106] with wpack[p, k*106+m] = CWT[128k+p, m]."""
    pad = N_KCHUNKS * KC - D_FEAT
    cw = np.concatenate(
        [CWT.astype(np.float32), np.zeros((pad, M_OUT), dtype=np.float32)], axis=0
    )  # [2176, 106]
    return np.ascontiguousarray(
        cw.reshape(N_KCHUNKS, KC, M_OUT).transpose(1, 0, 2).reshape(KC, -1)
    ).astype(dtype)


def _pack_x(XT16):
    """XT16 [2070, 8192] fp16 -> (xfat [128, 16*8192], xk16 [22, 8192]).

    xfat column order: for each block (c0, w), then k in 0..15, then c in
    0..w: xfat[p, 16*c0 + k*w + c] = XT16[128k+p, c0+c].
    """
    A = XT16[: NKF * KC].reshape(NKF, KC, N_PER_CORE)
    parts = []
    for c0, w in zip([sum(BLOCKS[:i]) for i in range(len(BLOCKS))], BLOCKS):
        parts.append(
            A[:, :, c0 : c0 + w].transpose(1, 0, 2).reshape(KC, NKF * w)
        )
    xfat = np.ascontiguousarray(np.concatenate(parts, axis=1))
    xk16 = np.ascontiguousarray(XT16[NKF * KC :])
    return xfat, xk16


def _ensure_ntff_hook():
    """bass_utils' trace path hard-imports antenv.axon_hooks, which this image
    lacks; shim it so a BASS_TRACE env var (or trace=True) can't crash."""
    import types

    try:
        from antenv.axon_hooks import get_axon_ntff_profile_hook  # noqa: F401

        return
    except ImportError:
        pass
    try:
        import antenv
        from trn_agent_boot.trn_boot import _ntff_profile_via_ctypes

        hook = _ntff_profile_via_ctypes("/opt/axon/libaxon_pjrt.so")
    except Exception:
        antenv, hook = None, None
    mod = types.ModuleType("antenv.axon_hooks")
    _h = [hook]
    mod.set_axon_ntff_profile_hook = lambda h: _h.__setitem__(0, h)
    mod.get_axon_ntff_profile_hook = lambda: _h[0]
    sys.modules["antenv.axon_hooks"] = mod
    if antenv is not None:
        antenv.axon_hooks = mod


def _run_device(XT, CWT, trace=False):
    """XT [D_FEAT, N_SENT] f32, CWT [D_FEAT, 106] f32 -> YT [106, N_SENT] f32."""
    _ensure_ntff_hook()
    from concourse.bass_utils import run_bass_kernel_spmd

    nc = _build()

    wpack = _pack_weights(CWT, np.float16)
    XT16 = XT.astype(np.float16)
    in_maps = []
    for c in range(N_CORES):
        xfat, xk16 = _pack_x(XT16[:, c * N_PER_CORE : (c + 1) * N_PER_CORE])
        in_maps.append({"xfat": xfat, "xk16": xk16, "cwf": wpack})

    res = run_bass_kernel_spmd(nc, in_maps, list(range(N_CORES)), trace=trace)
    yt = np.concatenate(
        [res.results[c]["yt"] for c in range(N_CORES)], axis=1
    ).astype(np.float32)
    return yt, res


def kernel(X, Constraints, W, b, X_Scope, X_Rel, _trace=False, _res_out=None):
    X = np.asarray(X)
    Constraints = np.asarray(Constraints)
    W = np.asarray(W)
    b = np.asarray(b)
    X_Scope = np.asarray(X_Scope)
    X_Rel = np.asarray(X_Rel)

    N, D = X.shape
    B = X_Scope.shape[0]
    R = Constraints.shape[0]
    assert (N, D, R) == (N_SENT, D_FEAT, N_REL), (N, D, R)

    XT = np.ascontiguousarray(X.T)
    CWT = np.ascontiguousarray(
        np.concatenate([Constraints, W], axis=0).T.astype(np.float32)
    )

    YT, res = _run_device(XT, CWT, trace=_trace)
    if _res_out is not None:
        _res_out.append(res)

    S_all = YT[:N_REL]          # [53, N] scores for every relation
    P = YT[N_REL:]              # [53, N] per-sentence classifier projections

    # host downstream on [N, 53]-sized data (mirrors reference semantics)
    starts = X_Scope[:, 0].astype(np.int64)
    seg = np.searchsorted(starts, np.arange(N, dtype=np.int64), side="right") - 1
    rel = np.asarray(X_Rel)[seg]  # wraps for seg == -1, same as jnp
    s = S_all[rel, np.arange(N)].astype(np.float64)

    valid = seg >= 0
    segv = seg[valid]
    m = np.full(B, -np.inf)
    np.maximum.at(m, segv, s[valid])
    e = np.exp(s - np.where(valid, m[np.clip(seg, 0, B - 1)], np.inf))
    e = np.where(valid, e, 0.0)
    z = np.bincount(segv, weights=e[valid], minlength=B)
    zsafe = np.where(z == 0.0, 1.0, z)
    w = e / zsafe[np.clip(seg, 0, B - 1)]

    out = np.empty((B, N_REL), dtype=np.float64)
    Pw = P.astype(np.float64) * w[None, :]
    for j in range(N_REL):
        out[:, j] = np.bincount(segv, weights=Pw[j, valid], minlength=B)
    out += b.astype(np.float64)[None, :]
    return out.astype(np.float32)


# revision 20
# speedup vs baseline: 1.1040x; 1.1040x over previous
"""Bass/Trainium2 kernel for nn_CGRE_68719477510 (ragged_sequence).

Restructure: scores[i] = X[i] . Constraints[rel(bag(i))] and the classifier
out = bag @ W.T are both projections of X onto small [53, 2070] matrices.
So one device pass computes Y = [Constraints; W] @ X.T  ([106, N]) — the only
traffic proportional to X. The segment softmax + weighted sum then operate on
the projected [N, 53] rows (P = X @ W.T), never touching X again:
    out[bag] = sum_i softmax_i(S) * P[i]  ==  (sum_i w_i X_i) @ W.T
Sharding: split sentences N=65536 into 8 contiguous chunks of 8192 (one per
core); replicate the small combined weight. The ragged segment ops run on
host over the tiny [N, 53] projection.

Precision: X and [C; W] are shipped in fp16 (e5m10). fp16xfp16 products are
exact in the f32 PSUM accumulator, so the only noise is the input rounding
(~2^-11 relative), giving ~1.7e-3 final Frobenius error — well under the
2e-2 gate — at half the DMA traffic of an f32/bf16-pair encoding.

DMA shape: the HWDGE rings release roughly one trigger per completed
transfer (~1-2 us apart), so sustained bandwidth ~ bytes-per-trigger /
pacing — triggers must stay fat. The host packs the first 16 k-chunks of
each column block k-interleaved so ONE dma_start fetches a whole block
across all k as a contiguous [128, 16*w] slice (16 KB descriptors, ~2 MB
per trigger). Column blocks shrink toward the end (512...128) and the
final block is k-split so only ~128 columns of matmul+copy+writeback trail
the last X byte. The ragged k=16 chunk (22 rows) rides in one early tile.
"""

import sys

sys.path.insert(0, "/opt/trn_rl_repo")

import numpy as np

N_SENT = 65536
D_FEAT = 2070
N_REL = 53
N_CORES = 8
N_PER_CORE = N_SENT // N_CORES  # 8192
M_OUT = 2 * N_REL  # 106 rows: [Constraints; W]

KC = 128                      # contraction chunk (partition dim)
NKF = 16                      # full 128-row k chunks (plus one 22-row tail)
N_KCHUNKS = NKF + 1           # 17
K_TAIL = D_FEAT - NKF * KC    # 22

# column blocks per core; fat 1024-col blocks carry the bulk (16 KB dma
# descriptors per half, fewer per-block PE transition penalties), the tail
# shrinks so the post-stream chain covers only 128 columns. b2 is 512 so
# the k16 tile slots into the sync ring at the PE-start sweet spot.
BLOCKS = [1024, 512, 1024, 1024, 1024, 1024, 1024, 1024, 256, 128, 128]
assert sum(BLOCKS) == N_PER_CORE
MM_G = 512  # psum accumulation group width (one PSUM bank)

_CACHE = {}


def _build_fp16():
    import concourse.mybir as mybir
    from concourse import bacc
    from concourse.tile import TileContext

    F16 = mybir.dt.float16
    F32 = mybir.dt.float32

    nc = bacc.Bacc("TRN2", target_bir_lowering=False, debug=True)
    # k-interleaved bulk: xfat[p, 16*cum(b) + k*w + c] = X[128k+p, c0+c]
    xfat = nc.dram_tensor("xfat", [KC, NKF * N_PER_CORE], F16, kind="ExternalInput")
    # ragged k=16 chunk, plain layout
    xk16 = nc.dram_tensor("xk16", [K_TAIL, N_PER_CORE], F16, kind="ExternalInput")
    # weights packed on host: cwf[p, k*106+m] = CW[m, 128k+p] (zero-padded)
    cwf = nc.dram_tensor("cwf", [KC, N_KCHUNKS * M_OUT], F16, kind="ExternalInput")
    yt = nc.dram_tensor("yt", [M_OUT, N_PER_CORE], F16, kind="ExternalOutput")

    X_BUFS = 5

    starts = [sum(BLOCKS[:i]) for i in range(len(BLOCKS))]
    fstarts = [NKF * s for s in starts]

    with TileContext(nc) as tc:
        with (
            tc.tile_pool(name="w", bufs=1) as wpool,
            tc.tile_pool(name="k16", bufs=1) as kpool,
            tc.tile_pool(name="x", bufs=X_BUFS) as xpool,
            tc.tile_pool(name="out", bufs=1) as opool,
            tc.tile_pool(name="psum", bufs=4, space="PSUM") as ppool,
        ):
            # weights ride gpsimd SWDGE during the ramp so the fat HWDGE
            # stream carries only bulk X
            wt = wpool.tile([KC, N_KCHUNKS * M_OUT], F16, tag="w")
            nc.gpsimd.dma_start(out=wt[:, :], in_=cwf[:, :])
            k16t = kpool.tile([K_TAIL, N_PER_CORE], F16, tag="k16")

            # Each block is split in half across BOTH rings (k0-7 on sync,
            # k8-15 on scalar) so blocks arrive in uniform lockstep at one
            # block per ~5.2us instead of pairwise bursts — the PE then never
            # stalls mid-stream (each stall resets the clock ramp to 1.2GHz
            # for ~3us). The shared k16 tile is slotted mid-ring so it lands
            # ~36us in; every block's accumulation does its k16 matmul FIRST,
            # delaying the PE start until then: the PE runs gapless at full
            # p-state from there and finishes right behind the last block.
            xts = {}
            for bi, (f0, w) in enumerate(zip(fstarts, BLOCKS)):
                xt = xpool.tile([KC, NKF * 1024], F16, tag="x")
                half = NKF * w // 2
                nc.sync.dma_start(out=xt[:, :half], in_=xfat[:, f0 : f0 + half])
                nc.scalar.dma_start(
                    out=xt[:, half : NKF * w], in_=xfat[:, f0 + half : f0 + NKF * w]
                )
                xts[bi] = xt
                if bi == 1:
                    nc.sync.dma_start(out=k16t[:, :], in_=xk16[:, :])

            out_t = opool.tile([M_OUT, N_PER_CORE], F16, tag="out")

            for bi, (c0, w) in enumerate(zip(starts, BLOCKS)):
                xt = xts[bi]
                # each 512-col psum group accumulates in one PSUM bank; a
                # 1024-col dma block holds two groups
                for g0 in range(0, w, MM_G):
                    gw = min(MM_G, w - g0)
                    psum = ppool.tile([M_OUT, MM_G], F32, tag="ps")
                    ws = slice(NKF * M_OUT, N_KCHUNKS * M_OUT)
                    nc.tensor.matmul(
                        psum[:, :gw],
                        wt[:K_TAIL, ws],
                        k16t[:, c0 + g0 : c0 + g0 + gw],
                        start=True,
                        stop=False,
                    )
                    for k in range(NKF):
                        ws = slice(k * M_OUT, (k + 1) * M_OUT)
                        nc.tensor.matmul(
                            psum[:, :gw],
                            wt[:, ws],
                            xt[:, k * w + g0 : k * w + g0 + gw],
                            start=False,
                            stop=(k == NKF - 1),
                        )
                    nc.vector.tensor_copy(
                        out=out_t[:, c0 + g0 : c0 + g0 + gw], in_=psum[:, :gw]
                    )
                    # thin-desc SWDGE writeback per 512-col group: descs
                    # <= 1KB stripe across all 16 engines (fat write descs
                    # pin to 2 engines!); they drain right as the read
                    # stream ends. The last two 128-col blocks merge into
                    # one write so a single gen trails the final cast.
                    if bi == len(BLOCKS) - 2:
                        continue
                    wb0 = starts[-2] if bi == len(BLOCKS) - 1 else c0 + g0
                    nc.gpsimd.dma_start(
                        out=yt[:, wb0 : c0 + g0 + gw],
                        in_=out_t[:, wb0 : c0 + g0 + gw],
                    )

    nc.compile()
    return nc


def _build():
    if "fp16" not in _CACHE:
        _CACHE["fp16"] = _build_fp16()
    return _CACHE["fp16"]


def _pack_weights(CWT, dtype):
    """CWT [D_FEAT, 106] -> [128, 17*106] with wpack[p, k*106+m] = CWT[128k+p, m]."""
    pad = N_KCHUNKS * KC - D_FEAT
    cw = np.concatenate(
        [CWT.astype(np.float32), np.zeros((pad, M_OUT), dtype=np.float32)], axis=0
    )  # [2176, 106]
    return np.ascontiguousarray(
        cw.reshape(N_KCHUNKS, KC, M_OUT).transpose(1, 0, 2).reshape(KC, -1)
    ).astype(dtype)


def _pack_x(XT16):
    """XT16 [2070, 8192] fp16 -> (xfat [128, 16*8192], xk16 [22, 8192]).

    xfat column order: for each block (c0, w), then k in 0..15, then c in
    0..w: xfat[p, 16*c0 + k*w + c] = XT16[128k+p, c0+c].
    """
    A = XT16[: NKF * KC].reshape(NKF, KC, N_PER_CORE)
    parts = []
    for c0, w in zip([sum(BLOCKS[:i]) for i in range(len(BLOCKS))], BLOCKS):
        parts.append(
            A[:, :, c0 : c0 + w].transpose(1, 0, 2).reshape(KC, NKF * w)
        )
    xfat = np.ascontiguousarray(np.concatenate(parts, axis=1))
    xk16 = np.ascontiguousarray(XT16[NKF * KC :])
    return xfat, xk16


def _ensure_ntff_hook():
    """bass_utils' trace path hard-imports antenv.axon_hooks, which this image
    lacks; shim it so a BASS_TRACE env var (or trace=True) can't crash."""
    import types

    try:
        from antenv.axon_hooks import get_axon_ntff_profile_hook  # noqa: F401

        return
    except ImportError:
        pass
    try:
        import antenv
        from trn_agent_boot.trn_boot import _ntff_profile_via_ctypes

        hook = _ntff_profile_via_ctypes("/opt/axon/libaxon_pjrt.so")
    except Exception:
        antenv, hook = None, None
    mod = types.ModuleType("antenv.axon_hooks")
    _h = [hook]
    mod.set_axon_ntff_profile_hook = lambda h: _h.__setitem__(0, h)
    mod.get_axon_ntff_profile_hook = lambda: _h[0]
    sys.modules["antenv.axon_hooks"] = mod
    if antenv is not None:
        antenv.axon_hooks = mod


def _run_device(XT, CWT, trace=False):
    """XT [D_FEAT, N_SENT] f32, CWT [D_FEAT, 106] f32 -> YT [106, N_SENT] f32."""
    _ensure_ntff_hook()
    from concourse.bass_utils import run_bass_kernel_spmd

    nc = _build()

    wpack = _pack_weights(CWT, np.float16)
    XT16 = XT.astype(np.float16)
    in_maps = []
    for c in range(N_CORES):
        xfat, xk16 = _pack_x(XT16[:, c * N_PER_CORE : (c + 1) * N_PER_CORE])
        in_maps.append({"xfat": xfat, "xk16": xk16, "cwf": wpack})

    res = run_bass_kernel_spmd(nc, in_maps, list(range(N_CORES)), trace=trace)
    yt = np.concatenate(
        [res.results[c]["yt"] for c in range(N_CORES)], axis=1
    ).astype(np.float32)
    return yt, res


def kernel(X, Constraints, W, b, X_Scope, X_Rel, _trace=False, _res_out=None):
    X = np.asarray(X)
    Constraints = np.asarray(Constraints)
    W = np.asarray(W)
    b = np.asarray(b)
    X_Scope = np.asarray(X_Scope)
    X_Rel = np.asarray(X_Rel)

    N, D = X.shape
    B = X_Scope.shape[0]
    R = Constraints.shape[0]
    assert (N, D, R) == (N_SENT, D_FEAT, N_REL), (N, D, R)

    XT = np.ascontiguousarray(X.T)
    CWT = np.ascontiguousarray(
        np.concatenate([Constraints, W], axis=0).T.astype(np.float32)
    )

    YT, res = _run_device(XT, CWT, trace=_trace)
    if _res_out is not None:
        _res_out.append(res)

    S_all = YT[:N_REL]          # [53, N] scores for every relation
    P = YT[N_REL:]              # [53, N] per-sentence classifier projections

    # host downstream on [N, 53]-sized data (mirrors reference semantics)
    starts = X_Scope[:, 0].astype(np.int64)
    seg = np.searchsorted(starts, np.arange(N, dtype=np.int64), side="right") - 1
    rel = np.asarray(X_Rel)[seg]  # wraps for seg == -1, same as jnp
    s = S_all[rel, np.arange(N)].astype(np.float64)

    valid = seg >= 0
    segv = seg[valid]
    m = np.full(B, -np.inf)
    np.maximum.at(m, segv, s[valid])
    e = np.exp(s - np.where(valid, m[np.clip(seg, 0, B - 1)], np.inf))
    e = np.where(valid, e, 0.0)
    z = np.bincount(segv, weights=e[valid], minlength=B)
    zsafe = np.where(z == 0.0, 1.0, z)
    w = e / zsafe[np.clip(seg, 0, B - 1)]

    out = np.empty((B, N_REL), dtype=np.float64)
    Pw = P.astype(np.float64) * w[None, :]
    for j in range(N_REL):
        out[:, j] = np.bincount(segv, weights=Pw[j, valid], minlength=B)
    out += b.astype(np.float64)[None, :]
    return out.astype(np.float32)
